# revision 1
# baseline (speedup 1.0000x reference)
"""Trainium2 Bass kernel for grouped (4 kv-group) causal self-attention with
a 1024-wide sliding window, RoPE, fused QKV projection and output projection.

Problem shapes (hardcoded): B=2, T=2048, C=2048, H=16, G=4, HS=128, SWS=1024.

Sharding over the 8 NeuronCores: core = b*4 + g — data-parallel over the
batch (2) and tensor-parallel over the 4 kv groups. Each core computes its
group's QKV projection (768 cols), RoPE, the 4 query heads' sliding-window
attention, and a partial output projection against its group's 512 columns
of W_proj; the host sums the 4 group partials per batch element.

Per-core kernel design (all PE matmuls in float32r — full rate at free >= 256;
operand tensors are declared float32r so every producer rounds on write,
which the BIR verifier requires):
  - one fully interleaved loop over 256-token chunks: qkv projection ->
    RoPE -> v transpose -> 4 heads' attention -> output projection, with
    k/v kept in a rolling 5-chunk ring (sliding window = 4 chunks back)
  - everything lives transposed: x^T [C,T], qkv^T [cols,T], cos/sin^T [HS,T]
  - RoPE rotate-half = PE matmul against a +-1 permutation matrix, then two
    multiplies and an add on the vector engine (in place on qkv^T)
  - scores computed transposed (S^T[j,i] = k_j . q_i) so that:
      * exp runs on the scalar engine straight out of PSUM into SBUF
      * the softmax denominator is an all-ones matmul on the PE (broadcast
        across partitions for free), reciprocal + multiply on vector engine
      * P^T feeds the y^T matmul directly (v natural-layout as stationary)
  - sliding-window/causal masking: gpsimd affine_select zeroing the post-exp
    P^T tiles (only the tiles crossing the diagonal or the window edge)
  - output projection consumes y^T directly as the stationary operand.
"""

import numpy as np
from contextlib import ExitStack

import concourse.bass as bass
import concourse.mybir as mybir
import concourse.tile as tile
from concourse import bacc
from concourse.bass_utils import run_bass_kernel_spmd
from concourse.masks import make_identity

F32 = mybir.dt.float32
F32R = mybir.dt.float32r
FP8 = mybir.dt.float8e4
AF = mybir.ActivationFunctionType
ALU = mybir.AluOpType

B, T, C, HS, NQ, G = 2, 2048, 2048, 128, 4, 4
G_COLS = 768  # per group: 4*128 q cols + 128 k + 128 v
SWS = 1024
SCALE = 1.0 / float(np.sqrt(np.float32(HS)))


def build_attention_nc(CHUNK=256, CC=512, DT=F32R, reps=1, xbufs=2, pbufs=6,
                       ybufs=2, rbufs=3, obufs=4, rtbufs=4,
                       fF=5, fY=1, fD=1, fP=1, noden=False,
                       rope_dma=False, fp8_den=False, yd_bank=False,
                       dbg_den=False, den_scale=8.0):
    CT = C // 128          # 16 contraction tiles for the qkv projection
    NCH = T // CHUNK       # chunks
    JPC = CHUNK // 128     # 128-wide j-tiles per chunk
    RING = SWS // CHUNK + 1  # k/v chunks alive (window + current)

    CS_DT = F32 if DT == F32R else DT
    nc = bacc.Bacc("TRN2", target_bir_lowering=False, debug=False)
    xT = nc.dram_tensor("xT", [C, T], DT, kind="ExternalInput").ap()
    wqkvT = nc.dram_tensor("wqkvT", [C, G_COLS], DT, kind="ExternalInput").ap()
    cosT = nc.dram_tensor("cosT", [HS, T], CS_DT, kind="ExternalInput").ap()
    sinT = nc.dram_tensor("sinT", [HS, T], CS_DT, kind="ExternalInput").ap()
    wprojT = nc.dram_tensor("wprojT", [NQ * HS, C], DT, kind="ExternalInput").ap()
    outp = nc.dram_tensor("outp", [T, C], F32, kind="ExternalOutput").ap()
    if dbg_den:
        dbgd = nc.dram_tensor("dbgd", [NQ, 128, CHUNK], F32,
                              kind="ExternalOutput").ap()

    xT_r = xT.rearrange("(co p) t -> p co t", p=128)      # [128, 16, T]
    wq_r = wqkvT.rearrange("(co p) n -> p co n", p=128)   # [128, 16, 768]
    wp_r = wprojT.rearrange("(h p) c -> p h c", p=128)    # [128, 4, C]
    out_r = outp.rearrange("(to p) c -> p to c", p=128)   # [128, 16, C]

    with tile.TileContext(nc) as tc, ExitStack() as ctx:
        const = ctx.enter_context(tc.tile_pool(name="const", bufs=1))
        wpool = ctx.enter_context(tc.tile_pool(name="wpool", bufs=1))
        qkvp = ctx.enter_context(tc.tile_pool(name="qkvp", bufs=RING))
        vpool = ctx.enter_context(tc.tile_pool(name="vpool", bufs=RING))
        xpool = ctx.enter_context(tc.tile_pool(name="xpool", bufs=xbufs))
        cspool = ctx.enter_context(tc.tile_pool(name="cspool", bufs=2))
        rtmp = ctx.enter_context(tc.tile_pool(name="rtmp", bufs=rtbufs))
        ppool = ctx.enter_context(tc.tile_pool(name="ppool", bufs=pbufs))
        ypool = ctx.enter_context(tc.tile_pool(name="ypool", bufs=ybufs))
        rpool = ctx.enter_context(tc.tile_pool(name="rpool", bufs=rbufs))
        opool = ctx.enter_context(tc.tile_pool(name="opool", bufs=obufs))
        # PSUM: 8 banks total -> flow 3 + y 2 + den 1 + proj 2
        ps_flow = ctx.enter_context(tc.tile_pool(name="psF", bufs=fF, space="PSUM"))
        ps_y = ctx.enter_context(tc.tile_pool(name="psY", bufs=fY, space="PSUM"))
        ps_d = ctx.enter_context(tc.tile_pool(name="psD", bufs=fD, space="PSUM"))
        ps_p = ctx.enter_context(tc.tile_pool(name="psP", bufs=fP, space="PSUM"))

        # rotate-half permutation, transposed: protT[p, f] = Prot[f, p].
        # gpsimd builds the f32 version; a DVE copy rounds into DT (verifier
        # requires a rounding producer for fp32r matmul operands).
        protT_f = const.tile([128, 128], F32, tag="protT_f")
        nc.gpsimd.memset(protT_f[:], 0.0)
        nc.gpsimd.affine_select(protT_f[:], protT_f[:], pattern=[[-1, 128]],
                                compare_op=ALU.not_equal, fill=-1.0,
                                base=-64, channel_multiplier=1)
        nc.gpsimd.affine_select(protT_f[:], protT_f[:], pattern=[[-1, 128]],
                                compare_op=ALU.not_equal, fill=1.0,
                                base=64, channel_multiplier=1)
        protT = const.tile([128, 128], DT, tag="protT")
        nc.vector.tensor_copy(out=protT[:], in_=protT_f[:])

        ident_f = const.tile([128, 128], F32, tag="ident_f")
        make_identity(nc, ident_f[:])
        ident = const.tile([128, 128], DT, tag="ident")
        nc.vector.tensor_copy(out=ident[:], in_=ident_f[:])

        onesf_f = const.tile([128, 128], F32, tag="onesf_f")
        nc.vector.memset(onesf_f[:], 1.0)
        onesf = const.tile([128, 128], DT, tag="onesf")
        nc.vector.tensor_copy(out=onesf[:], in_=onesf_f[:])

        if fp8_den:
            # DoubleRow denominator weights: planes of 16.0 in fp8e4. P is
            # converted to fp8 scaled by 1/16 (TRN2 fp8e4 is IEEE-ish E4M3:
            # values >240 decode as inf, and exp(s) reaches ~1000 here);
            # the 16x weights undo the scaling exactly.
            ones8_f = const.tile([128, 256], F32, tag="ones8_f")
            nc.vector.memset(ones8_f[:], den_scale)
            ones8 = const.tile([128, 2, 128], FP8, tag="ones8")
            nc.vector.tensor_copy(out=ones8[:], in_=ones8_f[:])
            p8pool = ctx.enter_context(tc.tile_pool(name="p8pool", bufs=pbufs))

        for _rep in range(reps):
            # weights: qkv weight split into 8 DMA parts so the first
            # projection matmuls start as soon as part 0 lands; the proj
            # weight is queued later (first needed ~chunk 0's projection)
            w_sb = wpool.tile([128, CT, G_COLS], DT, tag="bigw")
            for wp8 in range(8):
                nc.sync.dma_start(w_sb[:, wp8 * 2:(wp8 + 1) * 2, :],
                                  wq_r[:, wp8 * 2:(wp8 + 1) * 2, :])
            wp_sb = wpool.tile([128, NQ, C], DT, tag="bigwp")

            ring_qkv = [None] * NCH
            ring_v = [None] * NCH

            for icx in range(NCH):
                i0 = icx * CHUNK
                tsl = slice(i0, i0 + CHUNK)
                # --- qkv projection for this chunk ---
                xt = xpool.tile([128, CT, CHUNK], DT, tag="xT")
                nc.sync.dma_start(xt[:, 0:8, :], xT_r[:, 0:8, tsl])
                nc.sync.dma_start(xt[:, 8:16, :], xT_r[:, 8:16, tsl])
                cost = cspool.tile([128, CHUNK], CS_DT, tag="cosT")
                nc.sync.dma_start(cost[:], cosT[:, tsl])
                sint = cspool.tile([128, CHUNK], CS_DT, tag="sinT")
                nc.sync.dma_start(sint[:], sinT[:, tsl])

                qkv_c = qkvp.tile([128, 6, CHUNK], DT, tag="qkvT")
                v_c = vpool.tile([128, JPC, HS], DT, tag="vnat")
                ring_qkv[icx] = qkv_c
                ring_v[icx] = v_c
                if icx == 0:
                    # queue proj weight behind chunk 0's inputs (4 parts) —
                    # first consumed by chunk 0's output projection
                    for wp4 in range(4):
                        nc.sync.dma_start(wp_sb[:, wp4, :], wp_r[:, wp4, :])

                for m in range(6):
                    ps = ps_flow.tile([128, CHUNK], F32, tag="flow")
                    for ck in range(CT):
                        nc.tensor.matmul(ps[:], w_sb[:, ck, m * 128:(m + 1) * 128],
                                         xt[:, ck, :],
                                         start=(ck == 0), stop=(ck == CT - 1))
                    nc.scalar.copy(out=qkv_c[:, m, :], in_=ps[:])
                # --- rope (in place) on q heads + k ---
                for h in range(5):
                    qsl = qkv_c[:, h, :]
                    if rope_dma:
                        # rotate-half via SBUF->SBUF DMA swap; the sign of the
                        # first half is folded into sinT (host negates rows
                        # 0:64 — sin_flip in shard_inputs)
                        rot = rtmp.tile([128, CHUNK], DT, tag="roperot")
                        nc.sync.dma_start(rot[0:64, :], qsl[64:128, :])
                        nc.sync.dma_start(rot[64:128, :], qsl[0:64, :])
                        tmp = rtmp.tile([128, CHUNK], F32, tag="ropetmp")
                        nc.gpsimd.tensor_mul(tmp[:], qsl, cost[:])
                        nc.vector.tensor_mul(qsl, rot[:], sint[:])
                        nc.vector.tensor_add(qsl, qsl, tmp[:])
                    else:
                        psr = ps_flow.tile([128, CHUNK], F32, tag="flow")
                        nc.tensor.matmul(psr[:], protT[:], qsl, start=True,
                                         stop=True)
                        tmp = rtmp.tile([128, CHUNK], F32, tag="ropetmp")
                        nc.gpsimd.tensor_mul(tmp[:], qsl, cost[:])
                        nc.vector.tensor_mul(qsl, psr[:], sint[:])
                        nc.vector.tensor_add(qsl, qsl, tmp[:])
                # --- v back to natural [t, d] layout ---
                for jt in range(JPC):
                    pst = ps_flow.tile([128, 128], DT, tag="flow")
                    nc.tensor.transpose(pst[:], qkv_c[:, 5, jt * 128:(jt + 1) * 128],
                                        ident[:])
                    nc.scalar.copy(out=v_c[:, jt, :], in_=pst[:])

                # --- attention for the 4 heads of this chunk ---
                jt_lo = max(0, i0 - (SWS - 1)) // 128
                jt_hi = (i0 + CHUNK - 1) // 128
                yt = ypool.tile([128, NQ, CHUNK], DT, tag="yTc")
                for h in range(NQ):
                    if yd_bank:
                        # y and denominator accumulate side by side in ONE
                        # PSUM bank — halves the PE's bank-cycling rate in
                        # the attention inner loop
                        psyd = ps_y.tile([128, 2 * CHUNK], F32, tag="y")
                        psy = psyd[:, 0:CHUNK]
                        psd = psyd[:, CHUNK:2 * CHUNK]
                    else:
                        psy = ps_y.tile([128, CHUNK], F32, tag="y")
                        psd = ps_d.tile([128, CHUNK], F32, tag="d")
                    # j-tiles are processed in pairs: both score matmuls land
                    # in one PSUM bank (sequential single-matmul groups), and
                    # one exp covers both halves — halves ACT's fixed costs.
                    for jp in range(jt_lo, jt_hi + 1, 2):
                        psS = ps_flow.tile([128, 2 * CHUNK], F32, tag="flow",
                                           name="psS")
                        pt = ppool.tile([128, 2 * CHUNK], DT, tag="PT", name="pt")
                        for js2 in range(2):
                            jt = jp + js2
                            jc, js = jt // JPC, jt % JPC
                            kT_t = ring_qkv[jc][:, 4, js * 128:(js + 1) * 128]
                            nc.tensor.matmul(
                                psS[:, js2 * CHUNK:(js2 + 1) * CHUNK],
                                kT_t, qkv_c[:, h, :], start=True, stop=True)
                        nc.scalar.activation(pt[:], psS[:], AF.Exp, scale=SCALE)
                        for js2 in range(2):
                            jt = jp + js2
                            jc, js = jt // JPC, jt % JPC
                            v_t = ring_v[jc][:, js, :]
                            pth = pt[:, js2 * CHUNK:(js2 + 1) * CHUNK]
                            off = jt * 128 - i0
                            if off >= 0:
                                # causal: keep iff f - p - off >= 0  (i >= j)
                                nc.gpsimd.affine_select(
                                    pth, pth, pattern=[[1, CHUNK]],
                                    compare_op=ALU.is_ge, fill=0.0,
                                    base=-off, channel_multiplier=-1)
                            base_e = off + SWS
                            if base_e < CHUNK:
                                # window edge: keep iff p - f + base_e > 0
                                nc.gpsimd.affine_select(
                                    pth, pth, pattern=[[-1, CHUNK]],
                                    compare_op=ALU.is_gt, fill=0.0,
                                    base=base_e, channel_multiplier=1)
                            first = jt == jt_lo
                            last = jt == jt_hi
                            nc.tensor.matmul(psy[:], v_t, pth,
                                             start=first, stop=last)
                            if noden:
                                # timing probe only: wrong results
                                if first:
                                    nc.tensor.matmul(psd[:], onesf[:], pth,
                                                     start=True, stop=True)
                            elif not fp8_den:
                                nc.tensor.matmul(psd[:], onesf[:], pth,
                                                 start=first, stop=last)
                        if fp8_den and not noden:
                            # pair denominator at 0.5 cycles/row: convert the
                            # masked pair to fp8e4 planes, one DoubleRow
                            # matmul contracts both j-tiles
                            ptf8 = p8pool.tile([128, 2, CHUNK], FP8,
                                               tag="ptf8", name="ptf8")
                            # scale into fp8e4's safe range (max normal 240),
                            # clamp as insurance against overflow->inf
                            nc.gpsimd.tensor_scalar(
                                ptf8[:], pt[:], 1.0 / den_scale, 239.0,
                                op0=ALU.mult, op1=ALU.min)
                            nc.tensor.matmul(
                                psd[:], ones8[:], ptf8[:],
                                start=(jp == jt_lo), stop=(jp + 1 == jt_hi),
                                perf_mode=mybir.MatmulPerfMode.DoubleRow)
                    if dbg_den and icx == 4:
                        dsb = rpool.tile([128, CHUNK], F32, tag="dbgd")
                        nc.scalar.copy(out=dsb[:], in_=psd[:])
                        nc.sync.dma_start(dbgd[h], dsb[:])
                    rec = rpool.tile([128, CHUNK], F32, tag="recip")
                    nc.vector.reciprocal(rec[:], psd[:])
                    nc.vector.tensor_mul(yt[:, h, :], psy[:], rec[:])
                # --- output projection for this chunk's rows ---
                for tt in range(JPC):
                    tg = icx * JPC + tt
                    for ccx in range(C // CC):
                        psp = ps_p.tile([128, CC], F32, tag="proj")
                        for h in range(NQ):
                            nc.tensor.matmul(psp[:],
                                             yt[:, h, tt * 128:(tt + 1) * 128],
                                             wp_sb[:, h, ccx * CC:(ccx + 1) * CC],
                                             start=(h == 0), stop=(h == NQ - 1))
                        ost = opool.tile([128, CC], F32, tag="ostg")
                        nc.vector.tensor_copy(out=ost[:], in_=psp[:])
                        nc.sync.dma_start(out_r[:, tg, ccx * CC:(ccx + 1) * CC], ost[:])

    nc.compile()
    return nc


def shard_inputs(x, cos, sin, W_attn, W_proj, np_dtype=np.float32,
                 cs_dtype=None, sin_flip=False):
    """Full inputs -> list of 8 per-core input dicts (core = b*4 + g)."""
    if cs_dtype is None:
        cs_dtype = np.float32 if np_dtype == np.float32 else np_dtype
    in_maps = []
    cosT = np.ascontiguousarray(np.asarray(cos, dtype=np.float32).T).astype(cs_dtype)
    sinT_f = np.ascontiguousarray(np.asarray(sin, dtype=np.float32).T)
    if sin_flip:
        sinT_f = sinT_f.copy()
        sinT_f[:HS // 2] *= -1.0
    sinT = sinT_f.astype(cs_dtype)
    x = np.asarray(x, dtype=np.float32)
    W_attn = np.asarray(W_attn, dtype=np.float32)
    W_proj = np.asarray(W_proj, dtype=np.float32)
    for b in range(B):
        xTb = np.ascontiguousarray(x[b].T).astype(np_dtype)
        for g in range(G):
            in_maps.append({
                "xT": xTb,
                "wqkvT": np.ascontiguousarray(
                    W_attn[g * G_COLS:(g + 1) * G_COLS].T).astype(np_dtype),
                "cosT": cosT,
                "sinT": sinT,
                "wprojT": np.ascontiguousarray(
                    W_proj[:, g * NQ * HS:(g + 1) * NQ * HS].T).astype(np_dtype),
            })
    return in_maps


def unshard_output(results):
    out = np.zeros((B, T, C), np.float32)
    for b in range(B):
        for g in range(G):
            out[b] += results[b * G + g]["outp"]
    return out


_NC_CACHE = {}

# production configuration: RoPE rotate-half via SBUF->SBUF DMA swap (saves
# 40 PE matmuls + PSUM pressure per rep; numerically identical to the matmul
# rotate). fp8_den (DoubleRow denominators) measured as a large regression on
# HW — the gpsimd fp8 converts dominate — so it stays off.
KERNEL_KW = dict(rope_dma=True)
SHARD_KW = dict(sin_flip=True)


def get_nc():
    if "nc" not in _NC_CACHE:
        _NC_CACHE["nc"] = build_attention_nc(**KERNEL_KW)
    return _NC_CACHE["nc"]


def kernel(x, cos, sin, W_attn, W_proj):
    in_maps = shard_inputs(x, cos, sin, W_attn, W_proj, **SHARD_KW)
    nc = get_nc()
    res = run_bass_kernel_spmd(nc, in_maps, core_ids=list(range(8)))
    return unshard_output(res.results)



# revision 26
# speedup vs baseline: 1.2382x; 1.2382x over previous
"""Trainium2 Bass kernel for grouped (4 kv-group) causal self-attention with
a 1024-wide sliding window, RoPE, fused QKV projection and output projection.

Problem shapes (hardcoded): B=2, T=2048, C=2048, H=16, G=4, HS=128, SWS=1024.

Sharding over the 8 NeuronCores: core = b*4 + g — data-parallel over the
batch (2) and tensor-parallel over the 4 kv groups. Each core computes its
group's QKV projection (768 cols), RoPE, the 4 query heads' sliding-window
attention, and a partial output projection against its group's 512 columns
of W_proj; the host sums the 4 group partials per batch element.

Per-core kernel design (production: bf16 operands end-to-end — same PE rate
as float32r at free >= 256, half the DMA/SBUF traffic, rel_err ~7e-3 vs the
2e-2 gate; fp32 PSUM accumulation throughout):
  - one fully interleaved loop over 256-token chunks: qkv projection ->
    RoPE -> v transpose -> 4 heads' attention -> output projection, with
    k/v kept in a rolling 5-chunk ring (sliding window = 4 chunks back)
  - everything lives transposed: x^T [C,T], qkv^T [cols,T], cos/sin^T [HS,T]
  - RoPE rotate-half = PE matmul against a +-1 permutation matrix, then two
    multiplies and an add on the vector engine (in place on qkv^T)
  - scores computed transposed (S^T[j,i] = k_j . q_i) so that:
      * exp runs on the scalar engine straight out of PSUM into SBUF
      * the softmax denominator is an all-ones matmul on the PE (broadcast
        across partitions for free), reciprocal + multiply on vector engine
      * P^T feeds the y^T matmul directly (v natural-layout as stationary)
  - sliding-window/causal masking: gpsimd affine_select zeroing the post-exp
    P^T tiles (only the tiles crossing the diagonal or the window edge)
  - output projection consumes y^T directly as the stationary operand.
"""

import numpy as np
from contextlib import ExitStack

import concourse.bass as bass
import concourse.mybir as mybir
import concourse.tile as tile
from concourse import bacc
from concourse.bass_utils import run_bass_kernel_spmd
from concourse.masks import make_identity

F32 = mybir.dt.float32
F32R = mybir.dt.float32r
FP8 = mybir.dt.float8e4
AF = mybir.ActivationFunctionType
ALU = mybir.AluOpType

B, T, C, HS, NQ, G = 2, 2048, 2048, 128, 4, 4
G_COLS = 768  # per group: 4*128 q cols + 128 k + 128 v
SWS = 1024
SCALE = 1.0 / float(np.sqrt(np.float32(HS)))


BF16 = mybir.dt.bfloat16


def build_attention_nc(CHUNK=256, CC=512, DT=F32R, reps=1, xbufs=2, pbufs=6,
                       ybufs=2, rbufs=3, obufs=4, rtbufs=4,
                       fF=5, fY=1, fD=1, fP=1, noden=False,
                       rope_dma=False, fp8_den=False, yd_bank=False,
                       dbg_den=False, den_scale=8.0, fast_recip=False,
                       qkv0_ck=False, out_bf16=False, kv_first=False,
                       edge_slice=False, dma_spread=False, den_batch=False):
    CT = C // 128          # 16 contraction tiles for the qkv projection
    NCH = T // CHUNK       # chunks
    JPC = CHUNK // 128     # 128-wide j-tiles per chunk
    RING = SWS // CHUNK + 1  # k/v chunks alive (window + current)

    CS_DT = F32 if DT == F32R else DT
    nc = bacc.Bacc("TRN2", target_bir_lowering=False, debug=False)
    xT = nc.dram_tensor("xT", [C, T], DT, kind="ExternalInput").ap()
    wqkvT = nc.dram_tensor("wqkvT", [C, G_COLS], DT, kind="ExternalInput").ap()
    cosT = nc.dram_tensor("cosT", [HS, T], CS_DT, kind="ExternalInput").ap()
    sinT = nc.dram_tensor("sinT", [HS, T], CS_DT, kind="ExternalInput").ap()
    wprojT = nc.dram_tensor("wprojT", [NQ * HS, C], DT, kind="ExternalInput").ap()
    OUT_DT = mybir.dt.bfloat16 if out_bf16 else F32
    outp = nc.dram_tensor("outp", [T, C], OUT_DT, kind="ExternalOutput").ap()
    if dbg_den:
        dbgd = nc.dram_tensor("dbgd", [NQ, 128, CHUNK], F32,
                              kind="ExternalOutput").ap()

    xT_r = xT.rearrange("(co p) t -> p co t", p=128)      # [128, 16, T]
    wq_r = wqkvT.rearrange("(co p) n -> p co n", p=128)   # [128, 16, 768]
    wp_r = wprojT.rearrange("(h p) c -> p h c", p=128)    # [128, 4, C]
    out_r = outp.rearrange("(to p) c -> p to c", p=128)   # [128, 16, C]

    with tile.TileContext(nc) as tc, ExitStack() as ctx:
        const = ctx.enter_context(tc.tile_pool(name="const", bufs=1))
        wpool = ctx.enter_context(tc.tile_pool(name="wpool", bufs=1))
        qkvp = ctx.enter_context(tc.tile_pool(name="qkvp", bufs=RING))
        vpool = ctx.enter_context(tc.tile_pool(name="vpool", bufs=RING))
        xpool = ctx.enter_context(tc.tile_pool(name="xpool", bufs=xbufs))
        cspool = ctx.enter_context(tc.tile_pool(name="cspool", bufs=2))
        rtmp = ctx.enter_context(tc.tile_pool(name="rtmp", bufs=rtbufs))
        ppool = ctx.enter_context(tc.tile_pool(name="ppool", bufs=pbufs))
        ypool = ctx.enter_context(tc.tile_pool(name="ypool", bufs=ybufs))
        rpool = ctx.enter_context(tc.tile_pool(name="rpool", bufs=rbufs))
        opool = ctx.enter_context(tc.tile_pool(name="opool", bufs=obufs))
        # PSUM: 8 banks total -> flow 3 + y 2 + den 1 + proj 2
        ps_flow = ctx.enter_context(tc.tile_pool(name="psF", bufs=fF, space="PSUM"))
        ps_y = ctx.enter_context(tc.tile_pool(name="psY", bufs=fY, space="PSUM"))
        ps_d = ctx.enter_context(tc.tile_pool(name="psD", bufs=fD, space="PSUM"))
        ps_p = ctx.enter_context(tc.tile_pool(name="psP", bufs=fP, space="PSUM"))

        # rotate-half permutation, transposed: protT[p, f] = Prot[f, p].
        # gpsimd builds the f32 version; a DVE copy rounds into DT (verifier
        # requires a rounding producer for fp32r matmul operands).
        protT_f = const.tile([128, 128], F32, tag="protT_f")
        nc.gpsimd.memset(protT_f[:], 0.0)
        nc.gpsimd.affine_select(protT_f[:], protT_f[:], pattern=[[-1, 128]],
                                compare_op=ALU.not_equal, fill=-1.0,
                                base=-64, channel_multiplier=1)
        nc.gpsimd.affine_select(protT_f[:], protT_f[:], pattern=[[-1, 128]],
                                compare_op=ALU.not_equal, fill=1.0,
                                base=64, channel_multiplier=1)
        protT = const.tile([128, 128], DT, tag="protT")
        nc.vector.tensor_copy(out=protT[:], in_=protT_f[:])

        ident_f = const.tile([128, 128], F32, tag="ident_f")
        make_identity(nc, ident_f[:])
        ident = const.tile([128, 128], DT, tag="ident")
        nc.vector.tensor_copy(out=ident[:], in_=ident_f[:])

        onesf_f = const.tile([128, 128], F32, tag="onesf_f")
        nc.vector.memset(onesf_f[:], 1.0)
        onesf = const.tile([128, 128], DT, tag="onesf")
        nc.vector.tensor_copy(out=onesf[:], in_=onesf_f[:])

        if fp8_den:
            # DoubleRow denominator weights: planes of 16.0 in fp8e4. P is
            # converted to fp8 scaled by 1/16 (TRN2 fp8e4 is IEEE-ish E4M3:
            # values >240 decode as inf, and exp(s) reaches ~1000 here);
            # the 16x weights undo the scaling exactly.
            ones8_f = const.tile([128, 256], F32, tag="ones8_f")
            nc.vector.memset(ones8_f[:], den_scale)
            ones8 = const.tile([128, 2, 128], FP8, tag="ones8")
            nc.vector.tensor_copy(out=ones8[:], in_=ones8_f[:])
            p8pool = ctx.enter_context(tc.tile_pool(name="p8pool", bufs=pbufs))

        for _rep in range(reps):
            # weights: qkv weight split into 8 DMA parts so the first
            # projection matmuls start as soon as part 0 lands; the proj
            # weight is queued later (first needed ~chunk 0's projection)
            w_sb = wpool.tile([128, CT, G_COLS], DT, tag="bigw")
            w_parts = 1 if qkv0_ck else 8
            for wp8 in range(w_parts):
                nc.sync.dma_start(w_sb[:, wp8 * 2:(wp8 + 1) * 2, :],
                                  wq_r[:, wp8 * 2:(wp8 + 1) * 2, :])
            wp_sb = wpool.tile([128, NQ, C], DT, tag="bigwp")

            ring_qkv = [None] * NCH
            ring_v = [None] * NCH

            for icx in range(NCH):
                i0 = icx * CHUNK
                tsl = slice(i0, i0 + CHUNK)
                # --- qkv projection for this chunk ---
                xt = xpool.tile([128, CT, CHUNK], DT, tag="xT")
                nc.sync.dma_start(xt[:, 0:8, :], xT_r[:, 0:8, tsl])
                nc.sync.dma_start(xt[:, 8:16, :], xT_r[:, 8:16, tsl])
                cost = cspool.tile([128, CHUNK], CS_DT, tag="cosT")
                nc.sync.dma_start(cost[:], cosT[:, tsl])
                sint = cspool.tile([128, CHUNK], CS_DT, tag="sinT")
                nc.sync.dma_start(sint[:], sinT[:, tsl])

                qkv_c = qkvp.tile([128, 6, CHUNK], DT, tag="qkvT")
                v_c = vpool.tile([128, JPC, HS], DT, tag="vnat")
                ring_qkv[icx] = qkv_c
                ring_v[icx] = v_c
                if icx == 0:
                    # rest of the qkv weight (parts 1-7) behind chunk 0's
                    # x/cos/sin when chunk 0 runs ck-major
                    for wp8 in range(w_parts, 8):
                        nc.sync.dma_start(w_sb[:, wp8 * 2:(wp8 + 1) * 2, :],
                                          wq_r[:, wp8 * 2:(wp8 + 1) * 2, :])
                    # queue proj weight behind chunk 0's inputs (4 parts) —
                    # first consumed by chunk 0's output projection
                    for wp4 in range(4):
                        nc.sync.dma_start(wp_sb[:, wp4, :], wp_r[:, wp4, :])

                def rope_one(h):
                    # in-place rope on slot h (q heads 0-3, k at 4)
                    qsl = qkv_c[:, h, :]
                    if rope_dma:
                        # rotate-half via SBUF->SBUF DMA swap; the sign of the
                        # first half is folded into sinT (host negates rows
                        # 0:64 — sin_flip in shard_inputs)
                        rot = rtmp.tile([128, CHUNK], DT, tag="roperot")
                        rope_q = nc.scalar if dma_spread else nc.sync
                        rope_q.dma_start(rot[0:64, :], qsl[64:128, :])
                        rope_q.dma_start(rot[64:128, :], qsl[0:64, :])
                        tmp = rtmp.tile([128, CHUNK], DT, tag="ropetmp")
                        nc.gpsimd.tensor_mul(tmp[:], qsl, cost[:])
                        nc.vector.tensor_mul(qsl, rot[:], sint[:])
                        nc.vector.tensor_add(qsl, qsl, tmp[:])
                    else:
                        psr = ps_flow.tile([128, CHUNK], F32, tag="flow")
                        nc.tensor.matmul(psr[:], protT[:], qsl, start=True,
                                         stop=True)
                        tmp = rtmp.tile([128, CHUNK], F32, tag="ropetmp")
                        nc.gpsimd.tensor_mul(tmp[:], qsl, cost[:])
                        nc.vector.tensor_mul(qsl, psr[:], sint[:])
                        nc.vector.tensor_add(qsl, qsl, tmp[:])

                def vtrans():
                    # v back to natural [t, d] layout
                    for jt in range(JPC):
                        pst = ps_flow.tile([128, 128], DT, tag="flow")
                        nc.tensor.transpose(pst[:],
                                            qkv_c[:, 5, jt * 128:(jt + 1) * 128],
                                            ident[:])
                        nc.scalar.copy(out=v_c[:, jt, :], in_=pst[:])

                def post_one(m):
                    if m == 5:
                        vtrans()
                    else:
                        rope_one(m)

                if qkv0_ck and icx == 0:
                    # chunk 0 only: ck-major so the first matmuls need only
                    # w part 0 instead of the whole weight — hides the bulk
                    # of the initial weight DMA behind compute. Two passes of
                    # 3 single-bank accumulators (separate accumulation
                    # groups must not share a PSUM bank: start=True clears
                    # the whole bank).
                    halves = ([1, 0] if kv_first else [0, 1])
                    for half in halves:
                        ps3 = [ps_flow.tile([128, CHUNK], F32, tag="flow",
                                            name=f"qkv0_{half}_{i}")
                               for i in range(3)]
                        for ck in range(CT):
                            for i in range(3):
                                m = half * 3 + i
                                nc.tensor.matmul(
                                    ps3[i][:], w_sb[:, ck, m * 128:(m + 1) * 128],
                                    xt[:, ck, :],
                                    start=(ck == 0), stop=(ck == CT - 1))
                        for i in range(3):
                            nc.scalar.copy(out=qkv_c[:, half * 3 + i, :],
                                           in_=ps3[i][:])
                        for i in range(3):
                            post_one(half * 3 + i)
                else:
                    # k/v first so k's rope and v's transpose hide behind the
                    # four q projection groups
                    mm_order = [4, 5, 0, 1, 2, 3] if kv_first else list(range(6))
                    for m in mm_order:
                        ps = ps_flow.tile([128, CHUNK], F32, tag="flow")
                        for ck in range(CT):
                            nc.tensor.matmul(ps[:], w_sb[:, ck, m * 128:(m + 1) * 128],
                                             xt[:, ck, :],
                                             start=(ck == 0), stop=(ck == CT - 1))
                        nc.scalar.copy(out=qkv_c[:, m, :], in_=ps[:])
                        if kv_first:
                            post_one(m)
                    if not kv_first:
                        for h in range(5):
                            rope_one(h)
                        vtrans()

                # --- attention for the 4 heads of this chunk ---
                jt_lo = max(0, i0 - (SWS - 1)) // 128
                jt_hi = (i0 + CHUNK - 1) // 128
                yt = ypool.tile([128, NQ, CHUNK], DT, tag="yTc")
                for h in range(NQ):
                    if yd_bank:
                        # y and denominator accumulate side by side in ONE
                        # PSUM bank — halves the PE's bank-cycling rate in
                        # the attention inner loop
                        psyd = ps_y.tile([128, 2 * CHUNK], F32, tag="y")
                        psy = psyd[:, 0:CHUNK]
                        psd = psyd[:, CHUNK:2 * CHUNK]
                    else:
                        psy = ps_y.tile([128, CHUNK], F32, tag="y")
                        psd = ps_d.tile([128, CHUNK], F32, tag="d")
                    # j-tiles are processed in pairs: both score matmuls land
                    # in one PSUM bank (sequential single-matmul groups), and
                    # one exp covers both halves — halves ACT's fixed costs.
                    for jp in range(jt_lo, jt_hi + 1, 2):
                        psS = ps_flow.tile([128, 2 * CHUNK], F32, tag="flow",
                                           name="psS")
                        pt = ppool.tile([128, 2 * CHUNK], DT, tag="PT", name="pt")
                        for js2 in range(2):
                            jt = jp + js2
                            jc, js = jt // JPC, jt % JPC
                            kT_t = ring_qkv[jc][:, 4, js * 128:(js + 1) * 128]
                            off = jt * 128 - i0
                            if edge_slice:
                                # i columns with any live j in this tile:
                                # causal needs i >= jt*128, window needs
                                # i < jt*128+127 + SWS; stale psS regions are
                                # zeroed by the affine selects below
                                ilo = max(0, off)
                                ihi = min(CHUNK, off + SWS + 128)
                            else:
                                ilo, ihi = 0, CHUNK
                            nc.tensor.matmul(
                                psS[:, js2 * CHUNK + ilo:js2 * CHUNK + ihi],
                                kT_t, qkv_c[:, h, ilo:ihi], start=True, stop=True)
                        nc.scalar.activation(pt[:], psS[:], AF.Exp, scale=SCALE)
                        den_mms = []
                        for js2 in range(2):
                            jt = jp + js2
                            jc, js = jt // JPC, jt % JPC
                            v_t = ring_v[jc][:, js, :]
                            pth = pt[:, js2 * CHUNK:(js2 + 1) * CHUNK]
                            off = jt * 128 - i0
                            if edge_slice:
                                ilo = max(0, off)
                                ihi = min(CHUNK, off + SWS + 128)
                            else:
                                ilo, ihi = 0, CHUNK
                            if off >= 0:
                                # causal: keep iff f - p - off >= 0  (i >= j)
                                nc.gpsimd.affine_select(
                                    pth, pth, pattern=[[1, CHUNK]],
                                    compare_op=ALU.is_ge, fill=0.0,
                                    base=-off, channel_multiplier=-1)
                            base_e = off + SWS
                            if base_e < CHUNK:
                                # window edge: keep iff p - f + base_e > 0
                                nc.gpsimd.affine_select(
                                    pth, pth, pattern=[[-1, CHUNK]],
                                    compare_op=ALU.is_gt, fill=0.0,
                                    base=base_e, channel_multiplier=1)
                            first = jt == jt_lo
                            last = jt == jt_hi
                            nc.tensor.matmul(psy[:, ilo:ihi], v_t,
                                             pt[:, js2 * CHUNK + ilo:
                                                 js2 * CHUNK + ihi],
                                             start=first, stop=last)
                            if noden:
                                # timing probe only: wrong results
                                if first:
                                    nc.tensor.matmul(psd[:], onesf[:], pth,
                                                     start=True, stop=True)
                            elif not fp8_den:
                                if den_batch:
                                    # defer so both halves' den matmuls issue
                                    # back-to-back (ones stationary loads once)
                                    den_mms.append((ilo, ihi, js2, first, last))
                                else:
                                    nc.tensor.matmul(psd[:, ilo:ihi], onesf[:],
                                                     pt[:, js2 * CHUNK + ilo:
                                                         js2 * CHUNK + ihi],
                                                     start=first, stop=last)
                        for ilo, ihi, js2, first, last in den_mms:
                            nc.tensor.matmul(psd[:, ilo:ihi], onesf[:],
                                             pt[:, js2 * CHUNK + ilo:
                                                 js2 * CHUNK + ihi],
                                             start=first, stop=last)
                        if fp8_den and not noden:
                            # pair denominator at 0.5 cycles/row: convert the
                            # masked pair to fp8e4 planes, one DoubleRow
                            # matmul contracts both j-tiles
                            ptf8 = p8pool.tile([128, 2, CHUNK], FP8,
                                               tag="ptf8", name="ptf8")
                            # scale into fp8e4's safe range (max normal 240),
                            # clamp as insurance against overflow->inf
                            nc.gpsimd.tensor_scalar(
                                ptf8[:], pt[:], 1.0 / den_scale, 239.0,
                                op0=ALU.mult, op1=ALU.min)
                            nc.tensor.matmul(
                                psd[:], ones8[:], ptf8[:],
                                start=(jp == jt_lo), stop=(jp + 1 == jt_hi),
                                perf_mode=mybir.MatmulPerfMode.DoubleRow)
                    if dbg_den and icx == 4:
                        dsb = rpool.tile([128, CHUNK], F32, tag="dbgd")
                        nc.scalar.copy(out=dsb[:], in_=psd[:])
                        nc.sync.dma_start(dbgd[h], dsb[:])
                    rec = rpool.tile([128, CHUNK], F32, tag="recip")
                    if fast_recip:
                        nc.vector.reciprocal_approx_fast(rec[:], psd[:])
                    else:
                        nc.vector.reciprocal(rec[:], psd[:])
                    nc.vector.tensor_mul(yt[:, h, :], psy[:], rec[:])
                # --- output projection for this chunk's rows ---
                for tt in range(JPC):
                    tg = icx * JPC + tt
                    for ccx in range(C // CC):
                        psp = ps_p.tile([128, CC], F32, tag="proj")
                        for h in range(NQ):
                            nc.tensor.matmul(psp[:],
                                             yt[:, h, tt * 128:(tt + 1) * 128],
                                             wp_sb[:, h, ccx * CC:(ccx + 1) * CC],
                                             start=(h == 0), stop=(h == NQ - 1))
                        ost = opool.tile([128, CC], OUT_DT, tag="ostg")
                        nc.vector.tensor_copy(out=ost[:], in_=psp[:])
                        out_q = nc.gpsimd if dma_spread else nc.sync
                        out_q.dma_start(out_r[:, tg, ccx * CC:(ccx + 1) * CC], ost[:])

    nc.compile()
    return nc


def shard_inputs(x, cos, sin, W_attn, W_proj, np_dtype=np.float32,
                 cs_dtype=None, sin_flip=False):
    """Full inputs -> list of 8 per-core input dicts (core = b*4 + g)."""
    if cs_dtype is None:
        cs_dtype = np.float32 if np_dtype == np.float32 else np_dtype
    in_maps = []
    cosT = np.ascontiguousarray(np.asarray(cos, dtype=np.float32).T).astype(cs_dtype)
    sinT_f = np.ascontiguousarray(np.asarray(sin, dtype=np.float32).T)
    if sin_flip:
        sinT_f = sinT_f.copy()
        sinT_f[:HS // 2] *= -1.0
    sinT = sinT_f.astype(cs_dtype)
    x = np.asarray(x, dtype=np.float32)
    W_attn = np.asarray(W_attn, dtype=np.float32)
    W_proj = np.asarray(W_proj, dtype=np.float32)
    for b in range(B):
        xTb = np.ascontiguousarray(x[b].T).astype(np_dtype)
        for g in range(G):
            in_maps.append({
                "xT": xTb,
                "wqkvT": np.ascontiguousarray(
                    W_attn[g * G_COLS:(g + 1) * G_COLS].T).astype(np_dtype),
                "cosT": cosT,
                "sinT": sinT,
                "wprojT": np.ascontiguousarray(
                    W_proj[:, g * NQ * HS:(g + 1) * NQ * HS].T).astype(np_dtype),
            })
    return in_maps


def unshard_output(results):
    out = np.zeros((B, T, C), np.float32)
    for b in range(B):
        for g in range(G):
            out[b] += np.asarray(results[b * G + g]["outp"], dtype=np.float32)
    return out


_NC_CACHE = {}

# production configuration (all measured on HW via the differential bench):
#  - rope_dma: rotate-half via SBUF->SBUF DMA swap (saves 40 PE matmuls)
#  - DT=bf16 end-to-end (halves DMA + SBUF, same PE rate, rel_err ~7e-3)
#  - fast_recip: reciprocal_approx_fast instead of the iterative DVE divide
#    (~18us on HW)
#  - fF=3/fY=2/fP=2 PSUM split: double-buffered proj evacuation + per-head
#    y banks (~40us vs fF=5/fP=1)
#  - qkv0_ck: chunk 0 runs ck-major so compute starts after w part 0 lands
#  - out_bf16: bf16 partial outputs, host accumulates in fp32
# fp8_den (DoubleRow denominators) measured as a large regression on HW —
# the gpsimd fp8 converts dominate — so it stays off.
import ml_dtypes
KERNEL_KW = dict(rope_dma=True, DT=BF16, fast_recip=True, fF=3, fY=2, fP=2,
                 qkv0_ck=True, out_bf16=True, edge_slice=True, den_batch=True)
SHARD_KW = dict(sin_flip=True, np_dtype=ml_dtypes.bfloat16)


def get_nc():
    if "nc" not in _NC_CACHE:
        _NC_CACHE["nc"] = build_attention_nc(**KERNEL_KW)
    return _NC_CACHE["nc"]


def kernel(x, cos, sin, W_attn, W_proj):
    in_maps = shard_inputs(x, cos, sin, W_attn, W_proj, **SHARD_KW)
    nc = get_nc()
    res = run_bass_kernel_spmd(nc, in_maps, core_ids=list(range(8)))
    return unshard_output(res.results)



# revision 41
# speedup vs baseline: 1.7789x; 1.4367x over previous
"""Trainium2 Bass kernel for grouped (4 kv-group) causal self-attention with
a 1024-wide sliding window, RoPE, fused QKV projection and output projection.

Problem shapes (hardcoded): B=2, T=2048, C=2048, H=16, G=4, HS=128, SWS=1024.

Sharding over the 8 NeuronCores: core = b*4 + g — data-parallel over the
batch (2) and tensor-parallel over the 4 kv groups. Each core computes its
group's QKV projection (768 cols), RoPE, the 4 query heads' sliding-window
attention, and a partial output projection against its group's 512 columns
of W_proj; the host sums the 4 group partials per batch element.

Per-core kernel design (production: bf16 operands end-to-end — same PE rate
as float32r at free >= 256, half the DMA/SBUF traffic, rel_err ~7e-3 vs the
2e-2 gate; fp32 PSUM accumulation throughout):
  - one fully interleaved loop over 256-token chunks: qkv projection ->
    RoPE -> v transpose -> 4 heads' attention -> output projection, with
    k/v kept in a rolling 5-chunk ring (sliding window = 4 chunks back)
  - everything lives transposed: x^T [C,T], qkv^T [cols,T], cos/sin^T [HS,T]
  - RoPE rotate-half = PE matmul against a +-1 permutation matrix, then two
    multiplies and an add on the vector engine (in place on qkv^T)
  - scores computed transposed (S^T[j,i] = k_j . q_i) so that:
      * exp runs on the scalar engine straight out of PSUM into SBUF
      * the softmax denominator is an all-ones matmul on the PE (broadcast
        across partitions for free), reciprocal + multiply on vector engine
      * P^T feeds the y^T matmul directly (v natural-layout as stationary)
  - sliding-window/causal masking: gpsimd affine_select zeroing the post-exp
    P^T tiles (only the tiles crossing the diagonal or the window edge)
  - output projection consumes y^T directly as the stationary operand.
"""

import numpy as np
from contextlib import ExitStack

import concourse.bass as bass
import concourse.mybir as mybir
import concourse.tile as tile
from concourse import bacc
from concourse.bass_utils import run_bass_kernel_spmd
from concourse.masks import make_identity

F32 = mybir.dt.float32
F32R = mybir.dt.float32r
FP8 = mybir.dt.float8e4
AF = mybir.ActivationFunctionType
ALU = mybir.AluOpType

B, T, C, HS, NQ, G = 2, 2048, 2048, 128, 4, 4
G_COLS = 768  # per group: 4*128 q cols + 128 k + 128 v
SWS = 1024
SCALE = 1.0 / float(np.sqrt(np.float32(HS)))


BF16 = mybir.dt.bfloat16


def build_attention_nc(CHUNK=256, CC=512, DT=F32R, reps=1, xbufs=2, pbufs=6,
                       ybufs=2, rbufs=3, obufs=4, rtbufs=4,
                       fF=5, fY=1, fD=1, fP=1, noden=False,
                       rope_dma=False, fp8_den=False, yd_bank=False,
                       dbg_den=False, den_scale=8.0, fast_recip=False,
                       qkv0_ck=False, out_bf16=False, kv_first=False,
                       edge_slice=False, dma_spread=False, den_batch=False,
                       XC=None, hpair=False, qkv_dve=False, split_exp=False):
    CT = C // 128          # 16 contraction tiles for the qkv projection
    XC = XC or CHUNK       # projection-chunk width (N of the qkv matmuls)
    PJ = XC // CHUNK       # attention chunks per projection chunk
    NPX = T // XC          # projection chunks
    JPX = XC // 128        # 128-wide j-tiles per projection chunk
    TPC = CHUNK // 128     # row-tiles per attention chunk (output proj)
    RING = SWS // XC + 1   # k/v projection chunks alive (window + current)

    CS_DT = F32 if DT == F32R else DT
    nc = bacc.Bacc("TRN2", target_bir_lowering=False, debug=False)
    xT = nc.dram_tensor("xT", [C, T], DT, kind="ExternalInput").ap()
    wqkvT = nc.dram_tensor("wqkvT", [C, G_COLS], DT, kind="ExternalInput").ap()
    cosT = nc.dram_tensor("cosT", [HS, T], CS_DT, kind="ExternalInput").ap()
    sinT = nc.dram_tensor("sinT", [HS, T], CS_DT, kind="ExternalInput").ap()
    wprojT = nc.dram_tensor("wprojT", [NQ * HS, C], DT, kind="ExternalInput").ap()
    OUT_DT = mybir.dt.bfloat16 if out_bf16 else F32
    outp = nc.dram_tensor("outp", [T, C], OUT_DT, kind="ExternalOutput").ap()
    if dbg_den:
        dbgd = nc.dram_tensor("dbgd", [NQ, 128, CHUNK], F32,
                              kind="ExternalOutput").ap()

    xT_r = xT.rearrange("(co p) t -> p co t", p=128)      # [128, 16, T]
    wq_r = wqkvT.rearrange("(co p) n -> p co n", p=128)   # [128, 16, 768]
    wp_r = wprojT.rearrange("(h p) c -> p h c", p=128)    # [128, 4, C]
    out_r = outp.rearrange("(to p) c -> p to c", p=128)   # [128, 16, C]

    with tile.TileContext(nc) as tc, ExitStack() as ctx:
        const = ctx.enter_context(tc.tile_pool(name="const", bufs=1))
        wpool = ctx.enter_context(tc.tile_pool(name="wpool", bufs=1))
        qkvp = ctx.enter_context(tc.tile_pool(name="qkvp", bufs=RING))
        vpool = ctx.enter_context(tc.tile_pool(name="vpool", bufs=RING))
        xpool = ctx.enter_context(tc.tile_pool(name="xpool", bufs=xbufs))
        cspool = ctx.enter_context(tc.tile_pool(name="cspool", bufs=2))
        rtmp = ctx.enter_context(tc.tile_pool(name="rtmp", bufs=rtbufs))
        ppool = ctx.enter_context(tc.tile_pool(name="ppool", bufs=pbufs))
        ypool = ctx.enter_context(tc.tile_pool(name="ypool", bufs=ybufs))
        rpool = ctx.enter_context(tc.tile_pool(name="rpool", bufs=rbufs))
        opool = ctx.enter_context(tc.tile_pool(name="opool", bufs=obufs))
        # PSUM: 8 banks total -> flow 3 + y 2 + den 1 + proj 2
        ps_flow = ctx.enter_context(tc.tile_pool(name="psF", bufs=fF, space="PSUM"))
        ps_y = ctx.enter_context(tc.tile_pool(name="psY", bufs=fY, space="PSUM"))
        ps_d = ctx.enter_context(tc.tile_pool(name="psD", bufs=fD, space="PSUM"))
        ps_p = ctx.enter_context(tc.tile_pool(name="psP", bufs=fP, space="PSUM"))

        # rotate-half permutation, transposed: protT[p, f] = Prot[f, p].
        # gpsimd builds the f32 version; a DVE copy rounds into DT (verifier
        # requires a rounding producer for fp32r matmul operands).
        protT_f = const.tile([128, 128], F32, tag="protT_f")
        nc.gpsimd.memset(protT_f[:], 0.0)
        nc.gpsimd.affine_select(protT_f[:], protT_f[:], pattern=[[-1, 128]],
                                compare_op=ALU.not_equal, fill=-1.0,
                                base=-64, channel_multiplier=1)
        nc.gpsimd.affine_select(protT_f[:], protT_f[:], pattern=[[-1, 128]],
                                compare_op=ALU.not_equal, fill=1.0,
                                base=64, channel_multiplier=1)
        protT = const.tile([128, 128], DT, tag="protT")
        nc.vector.tensor_copy(out=protT[:], in_=protT_f[:])

        ident_f = const.tile([128, 128], F32, tag="ident_f")
        make_identity(nc, ident_f[:])
        ident = const.tile([128, 128], DT, tag="ident")
        nc.vector.tensor_copy(out=ident[:], in_=ident_f[:])

        onesf_f = const.tile([128, 128], F32, tag="onesf_f")
        nc.vector.memset(onesf_f[:], 1.0)
        onesf = const.tile([128, 128], DT, tag="onesf")
        nc.vector.tensor_copy(out=onesf[:], in_=onesf_f[:])

        if fp8_den:
            # DoubleRow denominator weights: planes of 16.0 in fp8e4. P is
            # converted to fp8 scaled by 1/16 (TRN2 fp8e4 is IEEE-ish E4M3:
            # values >240 decode as inf, and exp(s) reaches ~1000 here);
            # the 16x weights undo the scaling exactly.
            ones8_f = const.tile([128, 256], F32, tag="ones8_f")
            nc.vector.memset(ones8_f[:], den_scale)
            ones8 = const.tile([128, 2, 128], FP8, tag="ones8")
            nc.vector.tensor_copy(out=ones8[:], in_=ones8_f[:])
            p8pool = ctx.enter_context(tc.tile_pool(name="p8pool", bufs=pbufs))

        for _rep in range(reps):
            # weights: qkv weight split into 8 DMA parts so the first
            # projection matmuls start as soon as part 0 lands; the proj
            # weight is queued later (first needed ~chunk 0's projection)
            w_sb = wpool.tile([128, CT, G_COLS], DT, tag="bigw")
            w_parts = 1 if qkv0_ck else 8
            for wp8 in range(w_parts):
                nc.sync.dma_start(w_sb[:, wp8 * 2:(wp8 + 1) * 2, :],
                                  wq_r[:, wp8 * 2:(wp8 + 1) * 2, :])
            wp_sb = wpool.tile([128, NQ, C], DT, tag="bigwp")

            ring_qkv = [None] * NPX
            ring_v = [None] * NPX

            for px in range(NPX):
                x0 = px * XC
                tsl = slice(x0, x0 + XC)
                # --- qkv projection for this projection chunk ---
                xt = xpool.tile([128, CT, XC], DT, tag="xT")
                nc.sync.dma_start(xt[:, 0:8, :], xT_r[:, 0:8, tsl])
                nc.sync.dma_start(xt[:, 8:16, :], xT_r[:, 8:16, tsl])
                cost = cspool.tile([128, XC], CS_DT, tag="cosT")
                nc.sync.dma_start(cost[:], cosT[:, tsl])
                sint = cspool.tile([128, XC], CS_DT, tag="sinT")
                nc.sync.dma_start(sint[:], sinT[:, tsl])

                qkv_c = qkvp.tile([128, 6, XC], DT, tag="qkvT")
                v_c = vpool.tile([128, JPX, HS], DT, tag="vnat")
                ring_qkv[px] = qkv_c
                ring_v[px] = v_c
                if px == 0:
                    # rest of the qkv weight (parts 1-7) behind chunk 0's
                    # x/cos/sin when chunk 0 runs ck-major
                    for wp8 in range(w_parts, 8):
                        nc.sync.dma_start(w_sb[:, wp8 * 2:(wp8 + 1) * 2, :],
                                          wq_r[:, wp8 * 2:(wp8 + 1) * 2, :])
                    # queue proj weight behind chunk 0's inputs (4 parts) —
                    # first consumed by chunk 0's output projection
                    for wp4 in range(4):
                        nc.sync.dma_start(wp_sb[:, wp4, :], wp_r[:, wp4, :])

                def rope_one(h):
                    # in-place rope on slot h (q heads 0-3, k at 4)
                    qsl = qkv_c[:, h, :]
                    if rope_dma:
                        # rotate-half via SBUF->SBUF DMA swap; the sign of the
                        # first half is folded into sinT (host negates rows
                        # 0:64 — sin_flip in shard_inputs)
                        rot = rtmp.tile([128, XC], DT, tag="roperot")
                        rope_q = nc.scalar if dma_spread else nc.sync
                        rope_q.dma_start(rot[0:64, :], qsl[64:128, :])
                        rope_q.dma_start(rot[64:128, :], qsl[0:64, :])
                        tmp = rtmp.tile([128, XC], DT, tag="ropetmp")
                        nc.gpsimd.tensor_mul(tmp[:], qsl, cost[:])
                        nc.vector.tensor_mul(qsl, rot[:], sint[:])
                        nc.vector.tensor_add(qsl, qsl, tmp[:])
                    else:
                        psr = ps_flow.tile([128, XC], F32, tag="flow")
                        nc.tensor.matmul(psr[:], protT[:], qsl, start=True,
                                         stop=True)
                        tmp = rtmp.tile([128, XC], F32, tag="ropetmp")
                        nc.gpsimd.tensor_mul(tmp[:], qsl, cost[:])
                        nc.vector.tensor_mul(qsl, psr[:], sint[:])
                        nc.vector.tensor_add(qsl, qsl, tmp[:])

                def vtrans():
                    # v back to natural [t, d] layout
                    for jt in range(JPX):
                        pst = ps_flow.tile([128, 128], DT, tag="flow")
                        nc.tensor.transpose(pst[:],
                                            qkv_c[:, 5, jt * 128:(jt + 1) * 128],
                                            ident[:])
                        if qkv_dve:
                            nc.vector.tensor_copy(out=v_c[:, jt, :], in_=pst[:])
                        else:
                            nc.scalar.copy(out=v_c[:, jt, :], in_=pst[:])

                def post_one(m):
                    if m == 5:
                        vtrans()
                    else:
                        rope_one(m)

                if qkv0_ck and px == 0:
                    # chunk 0 only: ck-major so the first matmuls need only
                    # w part 0 instead of the whole weight — hides the bulk
                    # of the initial weight DMA behind compute. Two passes of
                    # 3 single-bank accumulators (separate accumulation
                    # groups must not share a PSUM bank: start=True clears
                    # the whole bank).
                    halves = ([1, 0] if kv_first else [0, 1])
                    for half in halves:
                        ps3 = [ps_flow.tile([128, XC], F32, tag="flow",
                                            name=f"qkv0_{half}_{i}")
                               for i in range(3)]
                        for ck in range(CT):
                            for i in range(3):
                                m = half * 3 + i
                                nc.tensor.matmul(
                                    ps3[i][:], w_sb[:, ck, m * 128:(m + 1) * 128],
                                    xt[:, ck, :],
                                    start=(ck == 0), stop=(ck == CT - 1))
                        for i in range(3):
                            if qkv_dve:
                                nc.vector.tensor_copy(
                                    out=qkv_c[:, half * 3 + i, :], in_=ps3[i][:])
                            else:
                                nc.scalar.copy(out=qkv_c[:, half * 3 + i, :],
                                               in_=ps3[i][:])
                        for i in range(3):
                            post_one(half * 3 + i)
                else:
                    # k/v first so k's rope and v's transpose hide behind the
                    # four q projection groups
                    mm_order = [4, 5, 0, 1, 2, 3] if kv_first else list(range(6))
                    for m in mm_order:
                        ps = ps_flow.tile([128, XC], F32, tag="flow")
                        for ck in range(CT):
                            nc.tensor.matmul(ps[:], w_sb[:, ck, m * 128:(m + 1) * 128],
                                             xt[:, ck, :],
                                             start=(ck == 0), stop=(ck == CT - 1))
                        if qkv_dve:
                            nc.vector.tensor_copy(out=qkv_c[:, m, :], in_=ps[:])
                        else:
                            nc.scalar.copy(out=qkv_c[:, m, :], in_=ps[:])
                        if kv_first:
                            post_one(m)
                    if not kv_first:
                        for h in range(5):
                            rope_one(h)
                        vtrans()

                # --- attention + output proj per 256-wide sub-chunk ---
                for sub in range(PJ):
                  icx = px * PJ + sub
                  i0 = icx * CHUNK
                  jt_lo = max(0, i0 - (SWS - 1)) // 128
                  jt_hi = (i0 + CHUNK - 1) // 128
                  yt = ypool.tile([128, NQ, CHUNK], DT, tag="yTc")
                  for hp in (range(2) if hpair else []):
                    # head-pair variant: kT/v/ones stationaries load once per
                    # j-tile for both heads; y and den for the pair accumulate
                    # in ONE bank each — only the very first matmul into a
                    # bank carries start=True (whole-bank has_written clear),
                    # every later matmul uses start=False (overwrite-where-
                    # unset covers the sibling head's region).
                    h0 = hp * 2
                    psy2 = ps_y.tile([128, 2, CHUNK], F32, tag="y")
                    psd2 = ps_d.tile([128, 2, CHUNK], F32, tag="d")
                    for jt in range(jt_lo, jt_hi + 1):
                        jc, js = jt // JPX, jt % JPX
                        kT_t = ring_qkv[jc][:, 4, js * 128:(js + 1) * 128]
                        v_t = ring_v[jc][:, js, :]
                        off = jt * 128 - i0
                        if edge_slice:
                            ilo = max(0, off)
                            ihi = min(CHUNK, off + SWS + 128)
                        else:
                            ilo, ihi = 0, CHUNK
                        psS2 = ps_flow.tile([128, 2, CHUNK], F32, tag="flow",
                                            name="psS2")
                        pt2 = ppool.tile([128, 2, CHUNK], DT, tag="PT",
                                         name="pt2")
                        for hh in range(2):
                            nc.tensor.matmul(
                                psS2[:, hh, ilo:ihi], kT_t,
                                qkv_c[:, h0 + hh, sub * CHUNK + ilo:
                                      sub * CHUNK + ihi],
                                start=(hh == 0), stop=(hh == 1))
                        nc.scalar.activation(pt2[:], psS2[:], AF.Exp,
                                             scale=SCALE)
                        if off >= 0:
                            # causal: keep iff f - p - off >= 0 (both heads)
                            nc.gpsimd.affine_select(
                                pt2[:], pt2[:], pattern=[[0, 2], [1, CHUNK]],
                                compare_op=ALU.is_ge, fill=0.0,
                                base=-off, channel_multiplier=-1)
                        base_e = off + SWS
                        if base_e < CHUNK:
                            # window edge: keep iff p - f + base_e > 0
                            nc.gpsimd.affine_select(
                                pt2[:], pt2[:], pattern=[[0, 2], [-1, CHUNK]],
                                compare_op=ALU.is_gt, fill=0.0,
                                base=base_e, channel_multiplier=1)
                        for hh in range(2):
                            nc.tensor.matmul(
                                psy2[:, hh, ilo:ihi], v_t, pt2[:, hh, ilo:ihi],
                                start=(jt == jt_lo and hh == 0),
                                stop=(jt == jt_hi and hh == 1))
                        for hh in range(2):
                            nc.tensor.matmul(
                                psd2[:, hh, ilo:ihi], onesf[:],
                                pt2[:, hh, ilo:ihi],
                                start=(jt == jt_lo and hh == 0),
                                stop=(jt == jt_hi and hh == 1))
                    rec2 = rpool.tile([128, 2, CHUNK], F32, tag="recip")
                    if fast_recip:
                        nc.vector.reciprocal_approx_fast(rec2[:], psd2[:])
                    else:
                        nc.vector.reciprocal(rec2[:], psd2[:])
                    nc.vector.tensor_mul(yt[:, h0:h0 + 2, :], psy2[:], rec2[:])
                  for h in ([] if hpair else range(NQ)):
                    if yd_bank:
                        # y and denominator accumulate side by side in ONE
                        # PSUM bank — halves the PE's bank-cycling rate in
                        # the attention inner loop
                        psyd = ps_y.tile([128, 2 * CHUNK], F32, tag="y")
                        psy = psyd[:, 0:CHUNK]
                        psd = psyd[:, CHUNK:2 * CHUNK]
                    else:
                        psy = ps_y.tile([128, CHUNK], F32, tag="y")
                        psd = ps_d.tile([128, CHUNK], F32, tag="d")
                    # j-tiles are processed in pairs: both score matmuls land
                    # in one PSUM bank (sequential single-matmul groups), and
                    # one exp covers both halves — halves ACT's fixed costs.
                    for jp in range(jt_lo, jt_hi + 1, 2):
                        psS = ps_flow.tile([128, 2 * CHUNK], F32, tag="flow",
                                           name="psS")
                        pt = ppool.tile([128, 2 * CHUNK], DT, tag="PT", name="pt")
                        for js2 in range(2):
                            jt = jp + js2
                            jc, js = jt // JPX, jt % JPX
                            kT_t = ring_qkv[jc][:, 4, js * 128:(js + 1) * 128]
                            off = jt * 128 - i0
                            if edge_slice:
                                # i columns with any live j in this tile:
                                # causal needs i >= jt*128, window needs
                                # i < jt*128+127 + SWS; stale psS regions are
                                # zeroed by the affine selects below
                                ilo = max(0, off)
                                ihi = min(CHUNK, off + SWS + 128)
                            else:
                                ilo, ihi = 0, CHUNK
                            nc.tensor.matmul(
                                psS[:, js2 * CHUNK + ilo:js2 * CHUNK + ihi],
                                kT_t, qkv_c[:, h, sub * CHUNK + ilo:
                                             sub * CHUNK + ihi],
                                start=True, stop=True)
                            if split_exp:
                                nc.scalar.activation(
                                    pt[:, js2 * CHUNK:(js2 + 1) * CHUNK],
                                    psS[:, js2 * CHUNK:(js2 + 1) * CHUNK],
                                    AF.Exp, scale=SCALE)
                        if not split_exp:
                            nc.scalar.activation(pt[:], psS[:], AF.Exp,
                                                 scale=SCALE)
                        den_mms = []
                        for js2 in range(2):
                            jt = jp + js2
                            jc, js = jt // JPX, jt % JPX
                            v_t = ring_v[jc][:, js, :]
                            pth = pt[:, js2 * CHUNK:(js2 + 1) * CHUNK]
                            off = jt * 128 - i0
                            if edge_slice:
                                ilo = max(0, off)
                                ihi = min(CHUNK, off + SWS + 128)
                            else:
                                ilo, ihi = 0, CHUNK
                            if off >= 0:
                                # causal: keep iff f - p - off >= 0  (i >= j)
                                nc.gpsimd.affine_select(
                                    pth, pth, pattern=[[1, CHUNK]],
                                    compare_op=ALU.is_ge, fill=0.0,
                                    base=-off, channel_multiplier=-1)
                            base_e = off + SWS
                            if base_e < CHUNK:
                                # window edge: keep iff p - f + base_e > 0
                                nc.gpsimd.affine_select(
                                    pth, pth, pattern=[[-1, CHUNK]],
                                    compare_op=ALU.is_gt, fill=0.0,
                                    base=base_e, channel_multiplier=1)
                            first = jt == jt_lo
                            last = jt == jt_hi
                            nc.tensor.matmul(psy[:, ilo:ihi], v_t,
                                             pt[:, js2 * CHUNK + ilo:
                                                 js2 * CHUNK + ihi],
                                             start=first, stop=last)
                            if noden:
                                # timing probe only: wrong results
                                if first:
                                    nc.tensor.matmul(psd[:], onesf[:], pth,
                                                     start=True, stop=True)
                            elif not fp8_den:
                                if den_batch:
                                    # defer so both halves' den matmuls issue
                                    # back-to-back (ones stationary loads once)
                                    den_mms.append((ilo, ihi, js2, first, last))
                                else:
                                    nc.tensor.matmul(psd[:, ilo:ihi], onesf[:],
                                                     pt[:, js2 * CHUNK + ilo:
                                                         js2 * CHUNK + ihi],
                                                     start=first, stop=last)
                        for ilo, ihi, js2, first, last in den_mms:
                            nc.tensor.matmul(psd[:, ilo:ihi], onesf[:],
                                             pt[:, js2 * CHUNK + ilo:
                                                 js2 * CHUNK + ihi],
                                             start=first, stop=last)
                        if fp8_den and not noden:
                            # pair denominator at 0.5 cycles/row: convert the
                            # masked pair to fp8e4 planes, one DoubleRow
                            # matmul contracts both j-tiles
                            ptf8 = p8pool.tile([128, 2, CHUNK], FP8,
                                               tag="ptf8", name="ptf8")
                            # scale into fp8e4's safe range (max normal 240),
                            # clamp as insurance against overflow->inf
                            nc.gpsimd.tensor_scalar(
                                ptf8[:], pt[:], 1.0 / den_scale, 239.0,
                                op0=ALU.mult, op1=ALU.min)
                            nc.tensor.matmul(
                                psd[:], ones8[:], ptf8[:],
                                start=(jp == jt_lo), stop=(jp + 1 == jt_hi),
                                perf_mode=mybir.MatmulPerfMode.DoubleRow)
                    if dbg_den and icx == 4:
                        dsb = rpool.tile([128, CHUNK], F32, tag="dbgd")
                        nc.scalar.copy(out=dsb[:], in_=psd[:])
                        nc.sync.dma_start(dbgd[h], dsb[:])
                    rec = rpool.tile([128, CHUNK], F32, tag="recip")
                    if fast_recip:
                        nc.vector.reciprocal_approx_fast(rec[:], psd[:])
                    else:
                        nc.vector.reciprocal(rec[:], psd[:])
                    nc.vector.tensor_mul(yt[:, h, :], psy[:], rec[:])
                  # --- output projection for this sub-chunk's rows ---
                  for tt in range(TPC):
                    tg = icx * TPC + tt
                    for ccx in range(C // CC):
                        psp = ps_p.tile([128, CC], F32, tag="proj")
                        for h in range(NQ):
                            nc.tensor.matmul(psp[:],
                                             yt[:, h, tt * 128:(tt + 1) * 128],
                                             wp_sb[:, h, ccx * CC:(ccx + 1) * CC],
                                             start=(h == 0), stop=(h == NQ - 1))
                        ost = opool.tile([128, CC], OUT_DT, tag="ostg")
                        nc.vector.tensor_copy(out=ost[:], in_=psp[:])
                        out_q = nc.gpsimd if dma_spread else nc.sync
                        out_q.dma_start(out_r[:, tg, ccx * CC:(ccx + 1) * CC], ost[:])

    nc.compile()
    return nc


def shard_inputs(x, cos, sin, W_attn, W_proj, np_dtype=np.float32,
                 cs_dtype=None, sin_flip=False):
    """Full inputs -> list of 8 per-core input dicts (core = b*4 + g)."""
    if cs_dtype is None:
        cs_dtype = np.float32 if np_dtype == np.float32 else np_dtype
    in_maps = []
    cosT = np.ascontiguousarray(np.asarray(cos, dtype=np.float32).T).astype(cs_dtype)
    sinT_f = np.ascontiguousarray(np.asarray(sin, dtype=np.float32).T)
    if sin_flip:
        sinT_f = sinT_f.copy()
        sinT_f[:HS // 2] *= -1.0
    sinT = sinT_f.astype(cs_dtype)
    x = np.asarray(x, dtype=np.float32)
    W_attn = np.asarray(W_attn, dtype=np.float32)
    W_proj = np.asarray(W_proj, dtype=np.float32)
    for b in range(B):
        xTb = np.ascontiguousarray(x[b].T).astype(np_dtype)
        for g in range(G):
            in_maps.append({
                "xT": xTb,
                "wqkvT": np.ascontiguousarray(
                    W_attn[g * G_COLS:(g + 1) * G_COLS].T).astype(np_dtype),
                "cosT": cosT,
                "sinT": sinT,
                "wprojT": np.ascontiguousarray(
                    W_proj[:, g * NQ * HS:(g + 1) * NQ * HS].T).astype(np_dtype),
            })
    return in_maps


def unshard_output(results):
    out = np.zeros((B, T, C), np.float32)
    for b in range(B):
        for g in range(G):
            out[b] += np.asarray(results[b * G + g]["outp"], dtype=np.float32)
    return out


_NC_CACHE = {}

# production configuration (all measured on HW via the differential bench):
#  - rope_dma: rotate-half via SBUF->SBUF DMA swap (saves 40 PE matmuls)
#  - DT=bf16 end-to-end (halves DMA + SBUF, same PE rate, rel_err ~7e-3)
#  - fast_recip: reciprocal_approx_fast instead of the iterative DVE divide
#    (~18us on HW)
#  - fF=3/fY=2/fP=2 PSUM split: double-buffered proj evacuation + per-head
#    y banks (~40us vs fF=5/fP=1)
#  - qkv0_ck: chunk 0 runs ck-major so compute starts after w part 0 lands
#  - out_bf16: bf16 partial outputs, host accumulates in fp32
# fp8_den (DoubleRow denominators) measured as a large regression on HW —
# the gpsimd fp8 converts dominate — so it stays off.
import ml_dtypes
KERNEL_KW = dict(rope_dma=True, DT=BF16, fast_recip=True, fF=3, fY=2, fP=2,
                 qkv0_ck=True, out_bf16=True, edge_slice=True, den_batch=True)
SHARD_KW = dict(sin_flip=True, np_dtype=ml_dtypes.bfloat16)


def get_nc():
    if "nc" not in _NC_CACHE:
        _NC_CACHE["nc"] = build_attention_nc(**KERNEL_KW)
    return _NC_CACHE["nc"]


def kernel(x, cos, sin, W_attn, W_proj):
    in_maps = shard_inputs(x, cos, sin, W_attn, W_proj, **SHARD_KW)
    nc = get_nc()
    res = run_bass_kernel_spmd(nc, in_maps, core_ids=list(range(8)))
    return unshard_output(res.results)

